# revision 16
# baseline (speedup 1.0000x reference)
"""Trainium2 Bass kernel for nn_DiseaseClassifier (segment_reduce).

reference semantics:
    m = mask.astype(f32); counts = m.sum(0)
    pooled = einsum('brh,rd->bdh', x, m) / max(counts,1)
    h = einsum('bdh,dhk->bdk', pooled, W1) + b1
    hn = LN(h) * gamma + beta ; g = gelu_exact(hn)
    preds = einsum('bdk,dk->bd', g, W2) + b2 ; preds[counts==0] = 0

Key algebraic facts used:
  * LayerNorm is scale-invariant, so the 1/count pooling divisor cancels
    (when b1 != 0 we add counts*b1 to the un-normalized pool-matmul output,
    which keeps the invariance exact).
  * LayerNorm is also shift-invariant, so W1/b1 are centered over the k
    axis on the host (W1c = W1 - mean_k W1); then mean_k(h) == 0 exactly
    and the LN needs only the variance: bn_stats -> bn_aggr -> sqrt ->
    reciprocal -> gelu(scale=1/sd), with no mean subtraction anywhere.
  * b2 is added on the host; the counts==0 zeroing folds into W2/b2.

Distribution: batch dim sharded over 8 NeuronCores (512 rows each); all
parameters replicated.  Inside each core:
  phase A: pool-matmul.  Stationary = x tiles [(4b,29r)=116, 128h] (fp16,
           single plane), moving = 0/1 block-diag mask [116, 56=(14d,4j)]
           -> PSUM [128h, 56], evacuated (ScalarE/VectorE split) into
           pooledT pt[h, d, b] (fp16).
  phase B: per-disease fp16 matmul pt[128h,128b] x W1c[d][128h,384k]
           (6 h-chunks accumulated in PSUM), variance-only LN, ScalarE
           gelu(scale*h), then fp16 multiply with replicated W2 and an
           innermost-axis reduce on DVE.

The emission is software-pipelined: phase A of chunk c+1 (x DMA + pool
matmuls + evacuations) is interleaved between the phase-B subgroups of
chunk c, so the PE has independent work queued while each subgroup's
bn->sqrt->recip->gelu chain (a ~3-4us cross-engine latency) completes,
and each subgroup's W2-dot is deferred by one subgroup so it never
queues ahead of the next chain's bn_stats on the DVE.

Measured (axon TRN2, per core, drift-cancelled repeat-loop):
  v1 (bf16 hi/lo, serial emission): 263 us
  v2 (fp16 single plane, resident W2): 194 us
  ablations: x-DMA alone 63 us (399 GB/s), +phaseA 68 us, +phaseB MMs
  129 us (PE ~125 us busy, both phases near structural roofline).
"""

import os
import sys
import functools

for _p in ("/opt/trn_rl_repo", "/opt/pypackages"):
    if os.path.isdir(_p) and _p not in sys.path:
        sys.path.insert(0, _p)

import numpy as np

B, R, H, D = 4096, 29, 768, 14
K = H // 2            # 384
LN_EPS = 1e-5
NCORES = 8
BC = B // NCORES      # 512 batch rows per core
NCHUNK = BC // 128    # 4 chunks of 128 rows
NG = 32               # (4b,29r) groups per chunk
GBX = 4               # groups per x-DMA batch (786 KB per DMA)
NGB = NG // GBX       # 4 x-DMAs per chunk
HC = H // 128         # 6 contraction chunks
JR = 4 * R            # 116 partitions for the pool matmul
DJ = D * 4            # 56 moving columns of the pool matmul


def _install_walrus_patches():
    """This walrus build supports only ONE sem wait per instruction
    ("Too many sync wait commands").  Split Tile-assigned multi-waits onto
    same-engine NoOps placed right before the instruction, and do the same
    for the TileContext tail drain."""
    from concourse import tile as _tile
    from concourse import mybir
    from concourse.vector_clock import ScopedClock

    if getattr(_tile.TileContext, "_ant_wait_split_patch", False):
        return
    _orig_commit = _tile.TileContext._commit_instruction

    def _patched_commit(self, inst, lazy_reg_writes=True):
        si = getattr(inst, "sync_info", None)
        if si is not None and si.on_wait and len(si.on_wait) > 1:
            waits = list(si.on_wait)
            inst.sync_info = mybir.SyncInfo(
                on_wait=[waits[-1]], on_update=list(si.on_update or [])
            )
            for w in waits[:-1]:
                nop = mybir.InstNoOp(
                    name=self.nc.get_next_instruction_name(), ins=[], outs=[]
                )
                nop.engine = inst.engine
                nop.sync_info = mybir.SyncInfo(on_wait=[w], on_update=[])
                self._add_instruction(nop)
        return _orig_commit(self, inst, lazy_reg_writes)

    def _patched_drain_and_barrier(self, tick_clock, wait_clock):
        drain_inst = self.nc.sync.drain()
        wait_clock.add_sem_waits(
            drain_inst.ins, ScopedClock({None: tick_clock.global_clock})
        )
        si = drain_inst.ins.sync_info
        if si is not None and si.on_wait and len(si.on_wait) > 1:
            waits = list(si.on_wait)
            drain_inst.ins.sync_info = mybir.SyncInfo(
                on_wait=[waits[0]], on_update=list(si.on_update or [])
            )
            for w in waits[1:]:
                d2 = self.nc.sync.drain()
                d2.ins.sync_info = mybir.SyncInfo(on_wait=[w], on_update=[])
        self.nc.all_engine_barrier()
        assert self.sems is not None
        popped = self.nc._tile_sem_poison_stack.pop()
        assert popped is self._sem_poison
        self.nc.clear_and_free_semaphores(list(self.sems.allocated().values()))
        self.nc.all_engine_barrier()

    _tile.TileContext._commit_instruction = _patched_commit
    _tile.TileContext._drain_and_barrier = _patched_drain_and_barrier
    _tile.TileContext._ant_wait_split_patch = True


@functools.lru_cache(maxsize=8)
def build_nc(with_b1: bool = False, with_affine: bool = False, repeat: int = 1,
             variant: str = "full", SG: int = 2, EVR: int = 10, EVV: int = 2,
             DOTGP: int = 0):
    """Build the Bass program (identical on all 8 cores).

    EVV of every EVR phase-A evacuation copies go to VectorE, the rest to
    ScalarE (load balance between the two PSUM-capable engines)."""
    import concourse.bass as bass
    import concourse.mybir as mybir
    from concourse.tile import TileContext

    _install_walrus_patches()

    F32 = mybir.dt.float32
    F16 = mybir.dt.float16
    AF = mybir.ActivationFunctionType
    ALU = mybir.AluOpType

    nc = bass.Bass("TRN2", target_bir_lowering=False, debug=False,
                   num_devices=NCORES)

    x = nc.declare_dram_parameter("x", [NCHUNK, NGB, 128, GBX * H],
                                  F16, isOutput=False)
    mblk = nc.declare_dram_parameter("mblk", [JR, DJ], F16, isOutput=False)
    w1t = nc.declare_dram_parameter("w1t", [128, D, HC, K], F16, isOutput=False)
    w2r = nc.declare_dram_parameter("w2r", [128, D, K], F16, isOutput=False)
    if with_b1:
        b1x = nc.declare_dram_parameter("b1x", [1, D * K], F16, isOutput=False)
    if with_affine:
        garep = nc.declare_dram_parameter("garep", [128, D, K], F32, isOutput=False)
        berep = nc.declare_dram_parameter("berep", [128, D, K], F32, isOutput=False)
    out = nc.declare_dram_parameter("out", [128, NCHUNK * D], F32, isOutput=True)

    with TileContext(nc) as tc:
        with (
            tc.tile_pool(name="const", bufs=1) as constp,
            tc.tile_pool(name="xin", bufs=8) as xp,
            tc.tile_pool(name="aff", bufs=4) as affp,
            tc.tile_pool(name="gly", bufs=3) as gp,
            tc.tile_pool(name="st", bufs=3) as stp,
            tc.tile_pool(name="pg", bufs=3, space="PSUM") as pgp,
            tc.tile_pool(name="hp", bufs=5, space="PSUM") as hpp,
        ):
            mb = constp.tile([JR, DJ], F16, tag="mblk")
            nc.sync.dma_start(out=mb[:], in_=mblk[:])
            w1sb = constp.tile([128, D, HC, K], F16, tag="w1sb")
            nc.sync.dma_start(out=w1sb[:], in_=w1t[:])
            w2sb = constp.tile([128, D, K], F16, tag="w2sb")
            nc.sync.dma_start(out=w2sb[:], in_=w2r[:])
            # double-buffered pooledT: phase A of chunk c+1 fills one parity
            # while phase B of chunk c reads the other (no WAR hazard).
            pts = [constp.tile([128, HC, D, 128], F16, tag=f"pt{i}",
                               name=f"pt{i}") for i in range(2)]

            outsb = constp.tile([128, NCHUNK * D], F32, tag="outsb")
            epst = constp.tile([128, 1], F32, tag="epst")
            nc.vector.memset(epst[:], LN_EPS)
            zerot = constp.tile([128, 1], F32, tag="zerot")
            nc.vector.memset(zerot[:], 0.0)
            evac_idx = [0]
            if variant != "full":
                nc.vector.memset(outsb[:], 0.0)
            if with_b1:
                ones = constp.tile([1, 128], F16, tag="ones")
                nc.vector.memset(ones[:], 1.0)
                b1sb = constp.tile([1, D * K], F16, tag="b1sb")
                nc.sync.dma_start(out=b1sb[:], in_=b1x[:])

            def emit_A_batch(c, gb):
                """One x DMA batch + its pool matmuls + PSUM evacuations."""
                xt = xp.tile([128, GBX * H], F16, tag="xt")
                nc.sync.dma_start(out=xt[:], in_=x[c, gb])
                for gg in range(GBX):
                    if variant == "dma":
                        continue
                    g = gb * GBX + gg
                    pg = pgp.tile([128, HC * DJ], F32, tag="pg")
                    for hc in range(HC):
                        nc.tensor.matmul(
                            pg[:, hc * DJ:(hc + 1) * DJ],
                            lhsT=xt[0:JR,
                                    gg * H + hc * 128:gg * H + (hc + 1) * 128],
                            rhs=mb[:],
                            start=(hc == 0),
                            stop=(hc == HC - 1),
                        )
                    # evacuate [128,(hc,d,j)] -> pt[:, hc, d, 4g:4g+4]
                    src = pg.rearrange("p (hc d j) -> p hc d j", hc=HC, d=D)
                    dst = pts[c % 2][:, :, :, 4 * g:4 * g + 4]
                    if evac_idx[0] % EVR < EVV:
                        nc.vector.tensor_copy(dst, src)
                    else:
                        nc.scalar.copy(dst, src)
                    evac_idx[0] += 1

            def emit_B_sub(c, d0):
                """Phase-B subgroup: matmuls + LN chain + gelu; returns a
                closure emitting the deferred W2-dot (or None)."""
                ds = list(range(d0, min(d0 + SG, D)))
                nsg = len(ds)
                hps_l = []
                for i, d in enumerate(ds):
                    hps = hpp.tile([128, K], F32, tag="hps")
                    hps_l.append(hps)
                    pt = pts[c % 2]
                    for hc in range(HC):
                        nc.tensor.matmul(
                            hps[:],
                            lhsT=pt[:, hc, d, :],
                            rhs=w1sb[:, d, hc, :],
                            start=(hc == 0),
                            stop=(hc == HC - 1) and not with_b1,
                        )
                    if with_b1:
                        nc.tensor.matmul(
                            hps[:],
                            lhsT=ones[:],
                            rhs=b1sb[:, d * K:(d + 1) * K],
                            start=False,
                            stop=True,
                        )
                if variant == "mmonly":
                    return None
                gtw = gp.tile([128, nsg * K], F16, tag="gtw")
                tmpw = gp.tile([128, nsg * K], F16, tag="tmpw")
                if variant == "plaingelu":
                    # timing ablation: gelu+dot without the LN-stats chain
                    for i, d in enumerate(ds):
                        nc.scalar.activation(
                            gtw[:, i * K:(i + 1) * K], hps_l[i][:], AF.Gelu)
                else:
                    agW = stp.tile([128, nsg, 2], F32, tag="agW")
                    sdW = stp.tile([128, nsg], F32, tag="sdW")
                    rsW = stp.tile([128, nsg], F32, tag="rsW")
                    for i, d in enumerate(ds):
                        bnst = stp.tile([128, 6], F32, tag="bnst")
                        nc.vector.bn_stats(bnst[:], hps_l[i][:])
                        nc.vector.bn_aggr(agW[:, i, :], bnst[:])
                    # mean_k(h)==0 by W1 centering: sd = sqrt(var+eps)
                    nc.scalar.activation(
                        sdW[:], agW[:, :, 1], AF.Sqrt, bias=epst[:, 0:1],
                    )
                    nc.vector.reciprocal(rsW[:], sdW[:])
                    for i, d in enumerate(ds):
                        if not with_affine:
                            nc.scalar.activation(
                                gtw[:, i * K:(i + 1) * K], hps_l[i][:], AF.Gelu,
                                bias=zerot[:, 0:1], scale=rsW[:, i:i + 1],
                            )
                        else:
                            hn = affp.tile([128, K], F32, tag="hn")
                            gat = affp.tile([128, K], F32, tag="gat")
                            bet = affp.tile([128, K], F32, tag="bet")
                            nc.sync.dma_start(out=gat[:], in_=garep[:, d, :])
                            nc.sync.dma_start(out=bet[:], in_=berep[:, d, :])
                            nc.scalar.activation(
                                hn[:], hps_l[i][:], AF.Identity,
                                bias=zerot[:, 0:1], scale=rsW[:, i:i + 1],
                            )
                            nc.vector.tensor_tensor(hn[:], hn[:], gat[:], op=ALU.mult)
                            nc.vector.tensor_tensor(hn[:], hn[:], bet[:], op=ALU.add)
                            nc.scalar.activation(
                                gtw[:, i * K:(i + 1) * K], hn[:], AF.Gelu)
                if variant == "nodot":
                    return None

                def emit_dot():
                    eng = nc.gpsimd if DOTGP else nc.vector
                    eng.tensor_tensor(
                        tmpw[:], gtw[:], w2sb[:, d0:d0 + nsg, :], op=ALU.mult,
                    )
                    nc.vector.reduce_sum(
                        outsb[:, c * D + d0:c * D + d0 + nsg],
                        tmpw.rearrange("p (n k) -> p n k", n=nsg),
                        axis=mybir.AxisListType.X,
                    )
                return emit_dot

            import contextlib
            loop_cm = tc.For_i(0, repeat, 1) if repeat > 1 else contextlib.nullcontext()
            # map next-chunk A batches onto phase-B subgroup slots
            nslots = (D + SG - 1) // SG
            slot_batches = [[] for _ in range(nslots)]
            for gb in range(NGB):
                slot_batches[min(gb, nslots - 1)].append(gb)
            if variant not in ("dma", "pool"):
                # prime the rotated pipeline: chunk 0 phase A, pre-loop
                for gb in range(NGB):
                    emit_A_batch(0, gb)
            with loop_cm:
              for c in range(NCHUNK):
                  if variant in ("dma", "pool"):
                      for gb in range(NGB):
                          emit_A_batch(c, gb)
                      continue
                  cn = (c + 1) % NCHUNK  # wraps to chunk 0 of the next iter
                  deferred = None
                  for k, d0 in enumerate(range(0, D, SG)):
                      dot = emit_B_sub(c, d0)
                      for gb in slot_batches[k]:
                          emit_A_batch(cn, gb)
                      if deferred is not None:
                          deferred()
                      deferred = dot
                  if deferred is not None:
                      deferred()

            nc.sync.dma_start(out=out[:], in_=outsb[:])

    return nc


def _host_prep(region_features, mask, W1, b1, gamma, beta, W2, b2):
    f32 = np.float32
    f16 = np.float16
    x = np.asarray(region_features)
    mask = np.asarray(mask)
    counts = mask.astype(np.int64).sum(axis=0)           # [D]
    ind = (counts > 0).astype(f32)                       # [D]

    # block-diag raw 0/1 mask: [(j,r)=116, (d,j)=56]
    mblk = np.zeros((JR, DJ), dtype=f16)
    mf = mask.astype(f32)                                # [R, D]
    for j in range(4):
        mblk[j * R:(j + 1) * R, :].reshape(R, D, 4)[:, :, j] = mf
    # center W1 (and b1) over k: LayerNorm is shift invariant, and with
    # mean_k(W1c)=0 the matmul output has exactly zero k-mean.
    W1c = np.asarray(W1, dtype=np.float64)
    W1c = W1c - W1c.mean(axis=2, keepdims=True)
    # w1 transposed to [p, d, hc, k] with h = hc*128 + p
    w1t = np.ascontiguousarray(
        W1c.reshape(D, HC, 128, K).transpose(2, 0, 1, 3)
    ).astype(f16)
    w2eff = np.asarray(W2, dtype=f32) * ind[:, None]
    w2r = np.ascontiguousarray(
        np.broadcast_to(w2eff[None].astype(f16), (128, D, K)))
    b2eff = np.asarray(b2, dtype=f32) * ind               # added on host

    b1c = np.asarray(b1, dtype=np.float64)
    b1c = b1c - b1c.mean(axis=1, keepdims=True)
    with_b1 = bool(np.any(np.abs(b1c) > 0.0))
    b1x = ((b1c * counts.astype(np.float64)[:, None]).reshape(1, D * K)
           .astype(f16) if with_b1 else None)

    ga = np.asarray(gamma, dtype=f32)
    be = np.asarray(beta, dtype=f32)
    with_affine = bool(np.any(ga != 1.0) or np.any(be != 0.0))
    garep = berep = None
    if with_affine:
        garep = np.ascontiguousarray(np.broadcast_to(ga[None], (128, D, K)))
        berep = np.ascontiguousarray(np.broadcast_to(be[None], (128, D, K)))

    common = {"mblk": mblk, "w1t": w1t, "w2r": w2r}
    extra = {"b2eff": b2eff}
    if with_b1:
        common["b1x"] = b1x
    if with_affine:
        common["garep"] = garep
        common["berep"] = berep
    in_maps = []
    for i in range(NCORES):
        m = dict(common)
        # b = c*128 + (gb*GBX+gg)*4 + j ; contiguous DMA layout
        xs = x[i * BC:(i + 1) * BC].reshape(NCHUNK, NGB, GBX, 4, R, H)
        xt_ = xs.transpose(0, 1, 3, 4, 2, 5).reshape(NCHUNK, NGB, JR, GBX * H)
        xp_ = np.zeros((NCHUNK, NGB, 128, GBX * H), dtype=f16)
        xp_[:, :, 0:JR, :] = xt_.astype(f16)
        m["x"] = xp_
        in_maps.append(m)
    return in_maps, with_b1, with_affine, extra


def kernel(region_features, mask, W1, b1, gamma, beta, W2, b2):
    from concourse.bass_utils import run_bass_kernel_spmd

    in_maps, with_b1, with_affine, extra = _host_prep(
        region_features, mask, W1, b1, gamma, beta, W2, b2
    )
    nc = build_nc(with_b1, with_affine)
    res = run_bass_kernel_spmd(nc, in_maps, list(range(NCORES)))
    outs = []
    for r in res.results:
        o = r["out"].reshape(128, NCHUNK, D).transpose(1, 0, 2).reshape(BC, D)
        outs.append(o)
    full = np.concatenate(outs, axis=0) + extra["b2eff"][None, :]
    return np.ascontiguousarray(full.astype(np.float32))


# revision 17
# speedup vs baseline: 1.3532x; 1.3532x over previous
"""Trainium2 Bass kernel for nn_DiseaseClassifier (segment_reduce).

reference semantics:
    m = mask.astype(f32); counts = m.sum(0)
    pooled = einsum('brh,rd->bdh', x, m) / max(counts,1)
    h = einsum('bdh,dhk->bdk', pooled, W1) + b1
    hn = LN(h) * gamma + beta ; g = gelu_exact(hn)
    preds = einsum('bdk,dk->bd', g, W2) + b2 ; preds[counts==0] = 0

Key algebraic facts used:
  * LayerNorm is scale-invariant, so the 1/count pooling divisor cancels
    (when b1 != 0 we add counts*b1 to the un-normalized pool-matmul output,
    which keeps the invariance exact).
  * LayerNorm is also shift-invariant, so W1/b1 are centered over the k
    axis on the host (W1c = W1 - mean_k W1); then mean_k(h) == 0 exactly
    and the LN needs only the variance: bn_stats -> bn_aggr -> sqrt ->
    reciprocal -> gelu(scale=1/sd), with no mean subtraction anywhere.
  * b2 is added on the host; the counts==0 zeroing folds into W2/b2.

Distribution: batch dim sharded over 8 NeuronCores (512 rows each); all
parameters replicated.  Inside each core:
  phase A: pool-matmul.  Stationary = x tiles [(4b,29r)=116, 128h] (fp16,
           single plane), moving = 0/1 block-diag mask [116, 56=(14d,4j)]
           -> PSUM [128h, 56], evacuated (ScalarE/VectorE split) into
           pooledT pt[h, d, b] (fp16).
  phase B: per-disease fp16 matmul pt[128h,128b] x W1c[d][128h,384k]
           (6 h-chunks accumulated in PSUM), variance-only LN, ScalarE
           gelu(scale*h), then fp16 multiply with replicated W2 and an
           innermost-axis reduce on DVE.

The emission is software-pipelined: phase A of chunk c+1 (x DMA + pool
matmuls + evacuations) is interleaved between the phase-B subgroups of
chunk c, so the PE has independent work queued while each subgroup's
bn->sqrt->recip->gelu chain (a ~3-4us cross-engine latency) completes,
and each subgroup's W2-dot is deferred by one subgroup so it never
queues ahead of the next chain's bn_stats on the DVE.

Measured (axon TRN2, per core, drift-cancelled repeat-loop):
  v1 (bf16 hi/lo, serial emission): 263 us
  v2 (fp16 single plane, resident W2): 194 us
  ablations: x-DMA alone 63 us (399 GB/s), +phaseA 68 us, +phaseB MMs
  129 us (PE ~125 us busy, both phases near structural roofline).
"""

import os
import sys
import functools

for _p in ("/opt/trn_rl_repo", "/opt/pypackages"):
    if os.path.isdir(_p) and _p not in sys.path:
        sys.path.insert(0, _p)

import numpy as np

B, R, H, D = 4096, 29, 768, 14
K = H // 2            # 384
LN_EPS = 1e-5
NCORES = 8
BC = B // NCORES      # 512 batch rows per core
NCHUNK = BC // 128    # 4 chunks of 128 rows
NG = 32               # (4b,29r) groups per chunk
GBX = 4               # groups per x-DMA batch (786 KB per DMA)
NGB = NG // GBX       # 4 x-DMAs per chunk
HC = H // 128         # 6 contraction chunks
JR = 4 * R            # 116 partitions for the pool matmul
DJ = D * 4            # 56 moving columns of the pool matmul


def _install_walrus_patches():
    """This walrus build supports only ONE sem wait per instruction
    ("Too many sync wait commands").  Split Tile-assigned multi-waits onto
    same-engine NoOps placed right before the instruction, and do the same
    for the TileContext tail drain."""
    from concourse import tile as _tile
    from concourse import mybir
    from concourse.vector_clock import ScopedClock

    if getattr(_tile.TileContext, "_ant_wait_split_patch", False):
        return
    _orig_commit = _tile.TileContext._commit_instruction

    def _patched_commit(self, inst, lazy_reg_writes=True):
        si = getattr(inst, "sync_info", None)
        if si is not None and si.on_wait and len(si.on_wait) > 1:
            waits = list(si.on_wait)
            inst.sync_info = mybir.SyncInfo(
                on_wait=[waits[-1]], on_update=list(si.on_update or [])
            )
            for w in waits[:-1]:
                nop = mybir.InstNoOp(
                    name=self.nc.get_next_instruction_name(), ins=[], outs=[]
                )
                nop.engine = inst.engine
                nop.sync_info = mybir.SyncInfo(on_wait=[w], on_update=[])
                self._add_instruction(nop)
        return _orig_commit(self, inst, lazy_reg_writes)

    def _patched_drain_and_barrier(self, tick_clock, wait_clock):
        drain_inst = self.nc.sync.drain()
        wait_clock.add_sem_waits(
            drain_inst.ins, ScopedClock({None: tick_clock.global_clock})
        )
        si = drain_inst.ins.sync_info
        if si is not None and si.on_wait and len(si.on_wait) > 1:
            waits = list(si.on_wait)
            drain_inst.ins.sync_info = mybir.SyncInfo(
                on_wait=[waits[0]], on_update=list(si.on_update or [])
            )
            for w in waits[1:]:
                d2 = self.nc.sync.drain()
                d2.ins.sync_info = mybir.SyncInfo(on_wait=[w], on_update=[])
        self.nc.all_engine_barrier()
        assert self.sems is not None
        popped = self.nc._tile_sem_poison_stack.pop()
        assert popped is self._sem_poison
        self.nc.clear_and_free_semaphores(list(self.sems.allocated().values()))
        self.nc.all_engine_barrier()

    _tile.TileContext._commit_instruction = _patched_commit
    _tile.TileContext._drain_and_barrier = _patched_drain_and_barrier
    _tile.TileContext._ant_wait_split_patch = True


@functools.lru_cache(maxsize=8)
def build_nc(with_b1: bool = False, with_affine: bool = False, repeat: int = 1,
             variant: str = "full", SG: int = 2, EVR: int = 10, EVV: int = 10,
             DOTGP: int = 1):
    """Build the Bass program (identical on all 8 cores).

    EVV of every EVR phase-A evacuation copies go to VectorE, the rest to
    ScalarE (load balance between the two PSUM-capable engines)."""
    import concourse.bass as bass
    import concourse.mybir as mybir
    from concourse.tile import TileContext

    _install_walrus_patches()

    F32 = mybir.dt.float32
    F16 = mybir.dt.float16
    AF = mybir.ActivationFunctionType
    ALU = mybir.AluOpType

    nc = bass.Bass("TRN2", target_bir_lowering=False, debug=False,
                   num_devices=NCORES)

    x = nc.declare_dram_parameter("x", [NCHUNK, NGB, 128, GBX * H],
                                  F16, isOutput=False)
    mblk = nc.declare_dram_parameter("mblk", [JR, DJ], F16, isOutput=False)
    w1t = nc.declare_dram_parameter("w1t", [128, D, HC, K], F16, isOutput=False)
    w2r = nc.declare_dram_parameter("w2r", [128, D, K], F16, isOutput=False)
    if with_b1:
        b1x = nc.declare_dram_parameter("b1x", [1, D * K], F16, isOutput=False)
    if with_affine:
        garep = nc.declare_dram_parameter("garep", [128, D, K], F32, isOutput=False)
        berep = nc.declare_dram_parameter("berep", [128, D, K], F32, isOutput=False)
    out = nc.declare_dram_parameter("out", [128, NCHUNK * D], F32, isOutput=True)

    with TileContext(nc) as tc:
        with (
            tc.tile_pool(name="const", bufs=1) as constp,
            tc.tile_pool(name="xin", bufs=8) as xp,
            tc.tile_pool(name="aff", bufs=4) as affp,
            tc.tile_pool(name="gly", bufs=3) as gp,
            tc.tile_pool(name="st", bufs=3) as stp,
            tc.tile_pool(name="pg", bufs=3, space="PSUM") as pgp,
            tc.tile_pool(name="hp", bufs=5, space="PSUM") as hpp,
        ):
            mb = constp.tile([JR, DJ], F16, tag="mblk")
            nc.sync.dma_start(out=mb[:], in_=mblk[:])
            w1sb = constp.tile([128, D, HC, K], F16, tag="w1sb")
            nc.sync.dma_start(out=w1sb[:], in_=w1t[:])
            w2sb = constp.tile([128, D, K], F16, tag="w2sb")
            nc.sync.dma_start(out=w2sb[:], in_=w2r[:])
            # double-buffered pooledT: phase A of chunk c+1 fills one parity
            # while phase B of chunk c reads the other (no WAR hazard).
            pts = [constp.tile([128, HC, D, 128], F16, tag=f"pt{i}",
                               name=f"pt{i}") for i in range(2)]

            outsb = constp.tile([128, NCHUNK * D], F32, tag="outsb")
            epst = constp.tile([128, 1], F32, tag="epst")
            nc.vector.memset(epst[:], LN_EPS)
            zerot = constp.tile([128, 1], F32, tag="zerot")
            nc.vector.memset(zerot[:], 0.0)
            evac_idx = [0]
            if variant != "full":
                nc.vector.memset(outsb[:], 0.0)
            if with_b1:
                ones = constp.tile([1, 128], F16, tag="ones")
                nc.vector.memset(ones[:], 1.0)
                b1sb = constp.tile([1, D * K], F16, tag="b1sb")
                nc.sync.dma_start(out=b1sb[:], in_=b1x[:])

            def emit_A_batch(c, gb):
                """One x DMA batch + its pool matmuls + PSUM evacuations."""
                xt = xp.tile([128, GBX * H], F16, tag="xt")
                nc.sync.dma_start(out=xt[:], in_=x[c, gb])
                for gg in range(GBX):
                    if variant == "dma":
                        continue
                    g = gb * GBX + gg
                    pg = pgp.tile([128, HC * DJ], F32, tag="pg")
                    for hc in range(HC):
                        nc.tensor.matmul(
                            pg[:, hc * DJ:(hc + 1) * DJ],
                            lhsT=xt[0:JR,
                                    gg * H + hc * 128:gg * H + (hc + 1) * 128],
                            rhs=mb[:],
                            start=(hc == 0),
                            stop=(hc == HC - 1),
                        )
                    # evacuate [128,(hc,d,j)] -> pt[:, hc, d, 4g:4g+4]
                    src = pg.rearrange("p (hc d j) -> p hc d j", hc=HC, d=D)
                    dst = pts[c % 2][:, :, :, 4 * g:4 * g + 4]
                    if evac_idx[0] % EVR < EVV:
                        nc.vector.tensor_copy(dst, src)
                    else:
                        nc.scalar.copy(dst, src)
                    evac_idx[0] += 1

            def emit_B_sub(c, d0):
                """Phase-B subgroup: matmuls + LN chain + gelu; returns a
                closure emitting the deferred W2-dot (or None)."""
                ds = list(range(d0, min(d0 + SG, D)))
                nsg = len(ds)
                hps_l = []
                for i, d in enumerate(ds):
                    hps = hpp.tile([128, K], F32, tag="hps")
                    hps_l.append(hps)
                    pt = pts[c % 2]
                    for hc in range(HC):
                        nc.tensor.matmul(
                            hps[:],
                            lhsT=pt[:, hc, d, :],
                            rhs=w1sb[:, d, hc, :],
                            start=(hc == 0),
                            stop=(hc == HC - 1) and not with_b1,
                        )
                    if with_b1:
                        nc.tensor.matmul(
                            hps[:],
                            lhsT=ones[:],
                            rhs=b1sb[:, d * K:(d + 1) * K],
                            start=False,
                            stop=True,
                        )
                if variant == "mmonly":
                    return None
                gtw = gp.tile([128, nsg * K], F16, tag="gtw")
                tmpw = gp.tile([128, nsg * K], F16, tag="tmpw")
                if variant == "plaingelu":
                    # timing ablation: gelu+dot without the LN-stats chain
                    for i, d in enumerate(ds):
                        nc.scalar.activation(
                            gtw[:, i * K:(i + 1) * K], hps_l[i][:], AF.Gelu)
                else:
                    agW = stp.tile([128, nsg, 2], F32, tag="agW")
                    sdW = stp.tile([128, nsg], F32, tag="sdW")
                    rsW = stp.tile([128, nsg], F32, tag="rsW")
                    for i, d in enumerate(ds):
                        bnst = stp.tile([128, 6], F32, tag="bnst")
                        nc.vector.bn_stats(bnst[:], hps_l[i][:])
                        nc.vector.bn_aggr(agW[:, i, :], bnst[:])
                    # mean_k(h)==0 by W1 centering: sd = sqrt(var+eps)
                    nc.scalar.activation(
                        sdW[:], agW[:, :, 1], AF.Sqrt, bias=epst[:, 0:1],
                    )
                    nc.vector.reciprocal(rsW[:], sdW[:])
                    for i, d in enumerate(ds):
                        if not with_affine:
                            nc.scalar.activation(
                                gtw[:, i * K:(i + 1) * K], hps_l[i][:], AF.Gelu,
                                bias=zerot[:, 0:1], scale=rsW[:, i:i + 1],
                            )
                        else:
                            hn = affp.tile([128, K], F32, tag="hn")
                            gat = affp.tile([128, K], F32, tag="gat")
                            bet = affp.tile([128, K], F32, tag="bet")
                            nc.sync.dma_start(out=gat[:], in_=garep[:, d, :])
                            nc.sync.dma_start(out=bet[:], in_=berep[:, d, :])
                            nc.scalar.activation(
                                hn[:], hps_l[i][:], AF.Identity,
                                bias=zerot[:, 0:1], scale=rsW[:, i:i + 1],
                            )
                            nc.vector.tensor_tensor(hn[:], hn[:], gat[:], op=ALU.mult)
                            nc.vector.tensor_tensor(hn[:], hn[:], bet[:], op=ALU.add)
                            nc.scalar.activation(
                                gtw[:, i * K:(i + 1) * K], hn[:], AF.Gelu)
                if variant == "nodot":
                    return None

                def emit_dot():
                    eng = nc.gpsimd if DOTGP else nc.vector
                    eng.tensor_tensor(
                        tmpw[:], gtw[:], w2sb[:, d0:d0 + nsg, :], op=ALU.mult,
                    )
                    nc.vector.reduce_sum(
                        outsb[:, c * D + d0:c * D + d0 + nsg],
                        tmpw.rearrange("p (n k) -> p n k", n=nsg),
                        axis=mybir.AxisListType.X,
                    )
                return emit_dot

            import contextlib
            loop_cm = tc.For_i(0, repeat, 1) if repeat > 1 else contextlib.nullcontext()
            # map next-chunk A batches onto phase-B subgroup slots
            nslots = (D + SG - 1) // SG
            slot_batches = [[] for _ in range(nslots)]
            for gb in range(NGB):
                slot_batches[min(gb, nslots - 1)].append(gb)
            if variant not in ("dma", "pool"):
                # prime the rotated pipeline: chunk 0 phase A, pre-loop
                for gb in range(NGB):
                    emit_A_batch(0, gb)
            with loop_cm:
              for c in range(NCHUNK):
                  if variant in ("dma", "pool"):
                      for gb in range(NGB):
                          emit_A_batch(c, gb)
                      continue
                  cn = (c + 1) % NCHUNK  # wraps to chunk 0 of the next iter
                  deferred = None
                  for k, d0 in enumerate(range(0, D, SG)):
                      dot = emit_B_sub(c, d0)
                      for gb in slot_batches[k]:
                          emit_A_batch(cn, gb)
                      if deferred is not None:
                          deferred()
                      deferred = dot
                  if deferred is not None:
                      deferred()

            nc.sync.dma_start(out=out[:], in_=outsb[:])

    return nc


def _host_prep(region_features, mask, W1, b1, gamma, beta, W2, b2):
    f32 = np.float32
    f16 = np.float16
    x = np.asarray(region_features)
    mask = np.asarray(mask)
    counts = mask.astype(np.int64).sum(axis=0)           # [D]
    ind = (counts > 0).astype(f32)                       # [D]

    # block-diag raw 0/1 mask: [(j,r)=116, (d,j)=56]
    mblk = np.zeros((JR, DJ), dtype=f16)
    mf = mask.astype(f32)                                # [R, D]
    for j in range(4):
        mblk[j * R:(j + 1) * R, :].reshape(R, D, 4)[:, :, j] = mf
    # center W1 (and b1) over k: LayerNorm is shift invariant, and with
    # mean_k(W1c)=0 the matmul output has exactly zero k-mean.
    W1c = np.asarray(W1, dtype=np.float64)
    W1c = W1c - W1c.mean(axis=2, keepdims=True)
    # w1 transposed to [p, d, hc, k] with h = hc*128 + p
    w1t = np.ascontiguousarray(
        W1c.reshape(D, HC, 128, K).transpose(2, 0, 1, 3)
    ).astype(f16)
    w2eff = np.asarray(W2, dtype=f32) * ind[:, None]
    w2r = np.ascontiguousarray(
        np.broadcast_to(w2eff[None].astype(f16), (128, D, K)))
    b2eff = np.asarray(b2, dtype=f32) * ind               # added on host

    b1c = np.asarray(b1, dtype=np.float64)
    b1c = b1c - b1c.mean(axis=1, keepdims=True)
    with_b1 = bool(np.any(np.abs(b1c) > 0.0))
    b1x = ((b1c * counts.astype(np.float64)[:, None]).reshape(1, D * K)
           .astype(f16) if with_b1 else None)

    ga = np.asarray(gamma, dtype=f32)
    be = np.asarray(beta, dtype=f32)
    with_affine = bool(np.any(ga != 1.0) or np.any(be != 0.0))
    garep = berep = None
    if with_affine:
        garep = np.ascontiguousarray(np.broadcast_to(ga[None], (128, D, K)))
        berep = np.ascontiguousarray(np.broadcast_to(be[None], (128, D, K)))

    common = {"mblk": mblk, "w1t": w1t, "w2r": w2r}
    extra = {"b2eff": b2eff}
    if with_b1:
        common["b1x"] = b1x
    if with_affine:
        common["garep"] = garep
        common["berep"] = berep
    in_maps = []
    for i in range(NCORES):
        m = dict(common)
        # b = c*128 + (gb*GBX+gg)*4 + j ; contiguous DMA layout
        xs = x[i * BC:(i + 1) * BC].reshape(NCHUNK, NGB, GBX, 4, R, H)
        xt_ = xs.transpose(0, 1, 3, 4, 2, 5).reshape(NCHUNK, NGB, JR, GBX * H)
        xp_ = np.zeros((NCHUNK, NGB, 128, GBX * H), dtype=f16)
        xp_[:, :, 0:JR, :] = xt_.astype(f16)
        m["x"] = xp_
        in_maps.append(m)
    return in_maps, with_b1, with_affine, extra


def kernel(region_features, mask, W1, b1, gamma, beta, W2, b2):
    from concourse.bass_utils import run_bass_kernel_spmd

    in_maps, with_b1, with_affine, extra = _host_prep(
        region_features, mask, W1, b1, gamma, beta, W2, b2
    )
    nc = build_nc(with_b1, with_affine)
    res = run_bass_kernel_spmd(nc, in_maps, list(range(NCORES)))
    outs = []
    for r in res.results:
        o = r["out"].reshape(128, NCHUNK, D).transpose(1, 0, 2).reshape(BC, D)
        outs.append(o)
    full = np.concatenate(outs, axis=0) + extra["b2eff"][None, :]
    return np.ascontiguousarray(full.astype(np.float32))


# revision 19
# speedup vs baseline: 1.3606x; 1.0054x over previous
"""Trainium2 Bass kernel for nn_DiseaseClassifier (segment_reduce).

reference semantics:
    m = mask.astype(f32); counts = m.sum(0)
    pooled = einsum('brh,rd->bdh', x, m) / max(counts,1)
    h = einsum('bdh,dhk->bdk', pooled, W1) + b1
    hn = LN(h) * gamma + beta ; g = gelu_exact(hn)
    preds = einsum('bdk,dk->bd', g, W2) + b2 ; preds[counts==0] = 0

Key algebraic facts used:
  * LayerNorm is scale-invariant, so the 1/count pooling divisor cancels
    (when b1 != 0 we add counts*b1 to the un-normalized pool-matmul output,
    which keeps the invariance exact).
  * LayerNorm is also shift-invariant, so W1/b1 are centered over the k
    axis on the host (W1c = W1 - mean_k W1); then mean_k(h) == 0 exactly
    and the LN needs only the variance: bn_stats -> bn_aggr -> sqrt ->
    reciprocal -> gelu(scale=1/sd), with no mean subtraction anywhere.
  * b2 is added on the host; the counts==0 zeroing folds into W2/b2.

Distribution: batch dim sharded over 8 NeuronCores (512 rows each); all
parameters replicated.  Inside each core:
  phase A: pool-matmul.  Stationary = x tiles [(4b,29r)=116, 128h] (fp16,
           single plane), moving = 0/1 block-diag mask [116, 56=(14d,4j)]
           -> PSUM [128h, 56], evacuated (ScalarE/VectorE split) into
           pooledT pt[h, d, b] (fp16).
  phase B: per-disease fp16 matmul pt[128h,128b] x W1c[d][128h,384k]
           (6 h-chunks accumulated in PSUM), variance-only LN, ScalarE
           gelu(scale*h), then fp16 multiply with replicated W2 and an
           innermost-axis reduce on DVE.

The emission is software-pipelined: phase A of chunk c+1 (x DMA + pool
matmuls + evacuations) is interleaved between the phase-B subgroups of
chunk c, so the PE has independent work queued while each subgroup's
bn->sqrt->recip->gelu chain (a ~3-4us cross-engine latency) completes,
and each subgroup's W2-dot is deferred by one subgroup so it never
queues ahead of the next chain's bn_stats on the DVE.

Measured (axon TRN2, per core, drift-cancelled repeat-loop):
  v1 (bf16 hi/lo, serial emission): 263 us
  v2 (fp16 single plane, resident W2): 194 us
  ablations: x-DMA alone 63 us (399 GB/s), +phaseA 68 us, +phaseB MMs
  129 us (PE ~125 us busy, both phases near structural roofline).
"""

import os
import sys
import functools

for _p in ("/opt/trn_rl_repo", "/opt/pypackages"):
    if os.path.isdir(_p) and _p not in sys.path:
        sys.path.insert(0, _p)

import numpy as np

B, R, H, D = 4096, 29, 768, 14
K = H // 2            # 384
LN_EPS = 1e-5
NCORES = 8
BC = B // NCORES      # 512 batch rows per core
NCHUNK = BC // 128    # 4 chunks of 128 rows
NG = 32               # (4b,29r) groups per chunk
GBX = 4               # groups per x-DMA batch (786 KB per DMA)
NGB = NG // GBX       # 4 x-DMAs per chunk
HC = H // 128         # 6 contraction chunks
JR = 4 * R            # 116 partitions for the pool matmul
DJ = D * 4            # 56 moving columns of the pool matmul


def _install_walrus_patches():
    """This walrus build supports only ONE sem wait per instruction
    ("Too many sync wait commands").  Split Tile-assigned multi-waits onto
    same-engine NoOps placed right before the instruction, and do the same
    for the TileContext tail drain."""
    from concourse import tile as _tile
    from concourse import mybir
    from concourse.vector_clock import ScopedClock

    if getattr(_tile.TileContext, "_ant_wait_split_patch", False):
        return
    _orig_commit = _tile.TileContext._commit_instruction

    def _patched_commit(self, inst, lazy_reg_writes=True):
        si = getattr(inst, "sync_info", None)
        if si is not None and si.on_wait and len(si.on_wait) > 1:
            waits = list(si.on_wait)
            inst.sync_info = mybir.SyncInfo(
                on_wait=[waits[-1]], on_update=list(si.on_update or [])
            )
            for w in waits[:-1]:
                nop = mybir.InstNoOp(
                    name=self.nc.get_next_instruction_name(), ins=[], outs=[]
                )
                nop.engine = inst.engine
                nop.sync_info = mybir.SyncInfo(on_wait=[w], on_update=[])
                self._add_instruction(nop)
        return _orig_commit(self, inst, lazy_reg_writes)

    def _patched_drain_and_barrier(self, tick_clock, wait_clock):
        drain_inst = self.nc.sync.drain()
        wait_clock.add_sem_waits(
            drain_inst.ins, ScopedClock({None: tick_clock.global_clock})
        )
        si = drain_inst.ins.sync_info
        if si is not None and si.on_wait and len(si.on_wait) > 1:
            waits = list(si.on_wait)
            drain_inst.ins.sync_info = mybir.SyncInfo(
                on_wait=[waits[0]], on_update=list(si.on_update or [])
            )
            for w in waits[1:]:
                d2 = self.nc.sync.drain()
                d2.ins.sync_info = mybir.SyncInfo(on_wait=[w], on_update=[])
        self.nc.all_engine_barrier()
        assert self.sems is not None
        popped = self.nc._tile_sem_poison_stack.pop()
        assert popped is self._sem_poison
        self.nc.clear_and_free_semaphores(list(self.sems.allocated().values()))
        self.nc.all_engine_barrier()

    _tile.TileContext._commit_instruction = _patched_commit
    _tile.TileContext._drain_and_barrier = _patched_drain_and_barrier
    _tile.TileContext._ant_wait_split_patch = True


@functools.lru_cache(maxsize=8)
def build_nc(with_b1: bool = False, with_affine: bool = False, repeat: int = 1,
             variant: str = "full", SG: int = 2, EVR: int = 10, EVV: int = 10,
             DOTGP: int = 1, VSQ: int = 0):
    """Build the Bass program (identical on all 8 cores).

    EVV of every EVR phase-A evacuation copies go to VectorE, the rest to
    ScalarE (load balance between the two PSUM-capable engines)."""
    import concourse.bass as bass
    import concourse.mybir as mybir
    from concourse.tile import TileContext

    _install_walrus_patches()

    F32 = mybir.dt.float32
    F16 = mybir.dt.float16
    AF = mybir.ActivationFunctionType
    ALU = mybir.AluOpType

    nc = bass.Bass("TRN2", target_bir_lowering=False, debug=False,
                   num_devices=NCORES)

    x = nc.declare_dram_parameter("x", [NCHUNK, NGB, 128, GBX * H],
                                  F16, isOutput=False)
    mblk = nc.declare_dram_parameter("mblk", [JR, DJ], F16, isOutput=False)
    w1t = nc.declare_dram_parameter("w1t", [128, D, HC, K], F16, isOutput=False)
    w2r = nc.declare_dram_parameter("w2r", [128, D, K], F16, isOutput=False)
    if with_b1:
        b1x = nc.declare_dram_parameter("b1x", [1, D * K], F16, isOutput=False)
    if with_affine:
        garep = nc.declare_dram_parameter("garep", [128, D, K], F32, isOutput=False)
        berep = nc.declare_dram_parameter("berep", [128, D, K], F32, isOutput=False)
    out = nc.declare_dram_parameter("out", [128, NCHUNK * D], F32, isOutput=True)

    with TileContext(nc) as tc:
        with (
            tc.tile_pool(name="const", bufs=1) as constp,
            tc.tile_pool(name="xin", bufs=8) as xp,
            tc.tile_pool(name="aff", bufs=4) as affp,
            tc.tile_pool(name="gly", bufs=3) as gp,
            tc.tile_pool(name="st", bufs=3) as stp,
            tc.tile_pool(name="pg", bufs=3, space="PSUM") as pgp,
            tc.tile_pool(name="hp", bufs=5, space="PSUM") as hpp,
        ):
            mb = constp.tile([JR, DJ], F16, tag="mblk")
            nc.sync.dma_start(out=mb[:], in_=mblk[:])
            w1sb = constp.tile([128, D, HC, K], F16, tag="w1sb")
            nc.sync.dma_start(out=w1sb[:], in_=w1t[:])
            w2sb = constp.tile([128, D, K], F16, tag="w2sb")
            nc.sync.dma_start(out=w2sb[:], in_=w2r[:])
            # double-buffered pooledT: phase A of chunk c+1 fills one parity
            # while phase B of chunk c reads the other (no WAR hazard).
            pts = [constp.tile([128, HC, D, 128], F16, tag=f"pt{i}",
                               name=f"pt{i}") for i in range(2)]

            outsb = constp.tile([128, NCHUNK * D], F32, tag="outsb")
            epst = constp.tile([128, 1], F32, tag="epst")
            nc.vector.memset(epst[:], LN_EPS)
            zerot = constp.tile([128, 1], F32, tag="zerot")
            nc.vector.memset(zerot[:], 0.0)
            evac_idx = [0]
            if variant != "full":
                nc.vector.memset(outsb[:], 0.0)
            if with_b1:
                ones = constp.tile([1, 128], F16, tag="ones")
                nc.vector.memset(ones[:], 1.0)
                b1sb = constp.tile([1, D * K], F16, tag="b1sb")
                nc.sync.dma_start(out=b1sb[:], in_=b1x[:])

            def emit_A_batch(c, gb):
                """One x DMA batch + its pool matmuls + PSUM evacuations."""
                xt = xp.tile([128, GBX * H], F16, tag="xt")
                nc.sync.dma_start(out=xt[:], in_=x[c, gb])
                for gg in range(GBX):
                    if variant == "dma":
                        continue
                    g = gb * GBX + gg
                    pg = pgp.tile([128, HC * DJ], F32, tag="pg")
                    for hc in range(HC):
                        nc.tensor.matmul(
                            pg[:, hc * DJ:(hc + 1) * DJ],
                            lhsT=xt[0:JR,
                                    gg * H + hc * 128:gg * H + (hc + 1) * 128],
                            rhs=mb[:],
                            start=(hc == 0),
                            stop=(hc == HC - 1),
                        )
                    # evacuate [128,(hc,d,j)] -> pt[:, hc, d, 4g:4g+4]
                    src = pg.rearrange("p (hc d j) -> p hc d j", hc=HC, d=D)
                    dst = pts[c % 2][:, :, :, 4 * g:4 * g + 4]
                    if evac_idx[0] % EVR < EVV:
                        nc.vector.tensor_copy(dst, src)
                    else:
                        nc.scalar.copy(dst, src)
                    evac_idx[0] += 1

            def emit_B_sub(c, d0):
                """Phase-B subgroup: matmuls + LN chain + gelu; returns a
                closure emitting the deferred W2-dot (or None)."""
                ds = list(range(d0, min(d0 + SG, D)))
                nsg = len(ds)
                hps_l = []
                for i, d in enumerate(ds):
                    hps = hpp.tile([128, K], F32, tag="hps")
                    hps_l.append(hps)
                    pt = pts[c % 2]
                    for hc in range(HC):
                        nc.tensor.matmul(
                            hps[:],
                            lhsT=pt[:, hc, d, :],
                            rhs=w1sb[:, d, hc, :],
                            start=(hc == 0),
                            stop=(hc == HC - 1) and not with_b1,
                        )
                    if with_b1:
                        nc.tensor.matmul(
                            hps[:],
                            lhsT=ones[:],
                            rhs=b1sb[:, d * K:(d + 1) * K],
                            start=False,
                            stop=True,
                        )
                if variant == "mmonly":
                    return None
                gtw = gp.tile([128, nsg * K], F16, tag="gtw")
                tmpw = gp.tile([128, nsg * K], F16, tag="tmpw")
                if variant == "plaingelu":
                    # timing ablation: gelu+dot without the LN-stats chain
                    for i, d in enumerate(ds):
                        nc.scalar.activation(
                            gtw[:, i * K:(i + 1) * K], hps_l[i][:], AF.Gelu)
                else:
                    sdW = stp.tile([128, nsg], F32, tag="sdW")
                    rsW = stp.tile([128, nsg], F32, tag="rsW")
                    if VSQ:
                        # mean_k(h)==0 by W1 centering, so var = sumsq/K via
                        # one ScalarE Square+accumulate per disease (keeps
                        # the variance pass off the DVE entirely)
                        ssW = stp.tile([128, nsg], F32, tag="ssW")
                        scr = gp.tile([128, K], F16, tag="scr")
                        for i, d in enumerate(ds):
                            nc.scalar.activation(
                                scr[:], hps_l[i][:], AF.Square,
                                accum_out=ssW[:, i:i + 1],
                            )
                        nc.scalar.activation(
                            sdW[:], ssW[:], AF.Sqrt, scale=1.0 / K,
                            bias=epst[:, 0:1],
                        )
                    else:
                        agW = stp.tile([128, nsg, 2], F32, tag="agW")
                        for i, d in enumerate(ds):
                            bnst = stp.tile([128, 6], F32, tag="bnst")
                            nc.vector.bn_stats(bnst[:], hps_l[i][:])
                            nc.vector.bn_aggr(agW[:, i, :], bnst[:])
                        # mean_k(h)==0 by W1 centering: sd = sqrt(var+eps)
                        nc.scalar.activation(
                            sdW[:], agW[:, :, 1], AF.Sqrt, bias=epst[:, 0:1],
                        )
                    nc.vector.reciprocal(rsW[:], sdW[:])
                    for i, d in enumerate(ds):
                        if not with_affine:
                            nc.scalar.activation(
                                gtw[:, i * K:(i + 1) * K], hps_l[i][:], AF.Gelu,
                                bias=zerot[:, 0:1], scale=rsW[:, i:i + 1],
                            )
                        else:
                            hn = affp.tile([128, K], F32, tag="hn")
                            gat = affp.tile([128, K], F32, tag="gat")
                            bet = affp.tile([128, K], F32, tag="bet")
                            nc.sync.dma_start(out=gat[:], in_=garep[:, d, :])
                            nc.sync.dma_start(out=bet[:], in_=berep[:, d, :])
                            nc.scalar.activation(
                                hn[:], hps_l[i][:], AF.Identity,
                                bias=zerot[:, 0:1], scale=rsW[:, i:i + 1],
                            )
                            nc.vector.tensor_tensor(hn[:], hn[:], gat[:], op=ALU.mult)
                            nc.vector.tensor_tensor(hn[:], hn[:], bet[:], op=ALU.add)
                            nc.scalar.activation(
                                gtw[:, i * K:(i + 1) * K], hn[:], AF.Gelu)
                if variant == "nodot":
                    return None

                def emit_dot():
                    eng = nc.gpsimd if DOTGP else nc.vector
                    eng.tensor_tensor(
                        tmpw[:], gtw[:], w2sb[:, d0:d0 + nsg, :], op=ALU.mult,
                    )
                    nc.vector.reduce_sum(
                        outsb[:, c * D + d0:c * D + d0 + nsg],
                        tmpw.rearrange("p (n k) -> p n k", n=nsg),
                        axis=mybir.AxisListType.X,
                    )
                return emit_dot

            import contextlib
            loop_cm = tc.For_i(0, repeat, 1) if repeat > 1 else contextlib.nullcontext()
            # map next-chunk A batches onto phase-B subgroup slots
            nslots = (D + SG - 1) // SG
            slot_batches = [[] for _ in range(nslots)]
            for gb in range(NGB):
                slot_batches[min(gb, nslots - 1)].append(gb)
            if variant not in ("dma", "pool"):
                # prime the rotated pipeline: chunk 0 phase A, pre-loop
                for gb in range(NGB):
                    emit_A_batch(0, gb)
            with loop_cm:
              for c in range(NCHUNK):
                  if variant in ("dma", "pool"):
                      for gb in range(NGB):
                          emit_A_batch(c, gb)
                      continue
                  cn = (c + 1) % NCHUNK  # wraps to chunk 0 of the next iter
                  deferred = None
                  for k, d0 in enumerate(range(0, D, SG)):
                      dot = emit_B_sub(c, d0)
                      for gb in slot_batches[k]:
                          emit_A_batch(cn, gb)
                      if deferred is not None:
                          deferred()
                      deferred = dot
                  if deferred is not None:
                      deferred()

            nc.sync.dma_start(out=out[:], in_=outsb[:])

    return nc


def _host_prep(region_features, mask, W1, b1, gamma, beta, W2, b2):
    f32 = np.float32
    f16 = np.float16
    x = np.asarray(region_features)
    mask = np.asarray(mask)
    counts = mask.astype(np.int64).sum(axis=0)           # [D]
    ind = (counts > 0).astype(f32)                       # [D]

    # block-diag raw 0/1 mask: [(j,r)=116, (d,j)=56]
    mblk = np.zeros((JR, DJ), dtype=f16)
    mf = mask.astype(f32)                                # [R, D]
    for j in range(4):
        mblk[j * R:(j + 1) * R, :].reshape(R, D, 4)[:, :, j] = mf
    # center W1 (and b1) over k: LayerNorm is shift invariant, and with
    # mean_k(W1c)=0 the matmul output has exactly zero k-mean.
    W1c = np.asarray(W1, dtype=np.float64)
    W1c = W1c - W1c.mean(axis=2, keepdims=True)
    # w1 transposed to [p, d, hc, k] with h = hc*128 + p
    w1t = np.ascontiguousarray(
        W1c.reshape(D, HC, 128, K).transpose(2, 0, 1, 3)
    ).astype(f16)
    w2eff = np.asarray(W2, dtype=f32) * ind[:, None]
    w2r = np.ascontiguousarray(
        np.broadcast_to(w2eff[None].astype(f16), (128, D, K)))
    b2eff = np.asarray(b2, dtype=f32) * ind               # added on host

    b1c = np.asarray(b1, dtype=np.float64)
    b1c = b1c - b1c.mean(axis=1, keepdims=True)
    with_b1 = bool(np.any(np.abs(b1c) > 0.0))
    b1x = ((b1c * counts.astype(np.float64)[:, None]).reshape(1, D * K)
           .astype(f16) if with_b1 else None)

    ga = np.asarray(gamma, dtype=f32)
    be = np.asarray(beta, dtype=f32)
    with_affine = bool(np.any(ga != 1.0) or np.any(be != 0.0))
    garep = berep = None
    if with_affine:
        garep = np.ascontiguousarray(np.broadcast_to(ga[None], (128, D, K)))
        berep = np.ascontiguousarray(np.broadcast_to(be[None], (128, D, K)))

    common = {"mblk": mblk, "w1t": w1t, "w2r": w2r}
    extra = {"b2eff": b2eff}
    if with_b1:
        common["b1x"] = b1x
    if with_affine:
        common["garep"] = garep
        common["berep"] = berep
    in_maps = []
    for i in range(NCORES):
        m = dict(common)
        # b = c*128 + (gb*GBX+gg)*4 + j ; contiguous DMA layout
        xs = x[i * BC:(i + 1) * BC].reshape(NCHUNK, NGB, GBX, 4, R, H)
        xt_ = xs.transpose(0, 1, 3, 4, 2, 5).reshape(NCHUNK, NGB, JR, GBX * H)
        xp_ = np.zeros((NCHUNK, NGB, 128, GBX * H), dtype=f16)
        xp_[:, :, 0:JR, :] = xt_.astype(f16)
        m["x"] = xp_
        in_maps.append(m)
    return in_maps, with_b1, with_affine, extra


def kernel(region_features, mask, W1, b1, gamma, beta, W2, b2):
    from concourse.bass_utils import run_bass_kernel_spmd

    in_maps, with_b1, with_affine, extra = _host_prep(
        region_features, mask, W1, b1, gamma, beta, W2, b2
    )
    nc = build_nc(with_b1, with_affine)
    res = run_bass_kernel_spmd(nc, in_maps, list(range(NCORES)))
    outs = []
    for r in res.results:
        o = r["out"].reshape(128, NCHUNK, D).transpose(1, 0, 2).reshape(BC, D)
        outs.append(o)
    full = np.concatenate(outs, axis=0) + extra["b2eff"][None, :]
    return np.ascontiguousarray(full.astype(np.float32))
